# revision 1
# baseline (speedup 1.0000x reference)
"""Trainium2 Bass kernel for nn_CrossAttention (B=4, N=2048, C=1024, H=16).

Sharding: 8 cores = 4 batches x 2 query-stripe halves. Core (b, half)
computes all 16 heads for the 1024 query rows in the 128-row blocks
{2j+half : j=0..7} of batch b. Each core computes its batch's full K/V
projection (duplicated across the pair), so there is no cross-core
communication at all; outputs are disjoint row-slices of the final
projection.

Causality: query blocks are processed in pairs (256 rows, global blocks
4p+half and 4p+2+half). Pair p attends to key chunks 0..4p+3; the last
4 key chunks get a multiplicative mask that depends only on `half`, so
a single compiled kernel serves all 8 cores with the mask passed as
data.

Attention math (per head h, pair p):
  S^T[k,q] = sum_d kT[d,k] qhT[d,q]     (64-contraction matmuls; heads
             are processed in even/odd pairs so consecutive matmuls hit
             disjoint PE row groups and run concurrently)
  E = exp(0.125 * S^T)   (no max-subtraction; logits are O(1))
  E tail *= mask
  [Y^T; denom] = matmul(lhsT=[v|1] chunk, rhs=E)  -> psum [65, 256]
  yT = Y^T * broadcast(1/denom)   (gpsimd partition_broadcast)
Then out rows = yT^T @ projT + bias via natural matmuls, with the bias
added as a rank-1 accumulating matmul. yT is written into the storage
of qhT (each 256-column slice of qhT is dead once the same pair's
scores are done, so the attention output can reuse it in place).

Independent projection work (kv/q/out projections) is interleaved into
the attention stream as "background groups" popped between score/AV
steps, so the PE stays busy while ScalarE computes exponentials.
"""

import collections

import numpy as np
import ml_dtypes

import concourse.bass as bass
import concourse.tile as tile
from concourse import bacc, mybir

P = 128
B, N, C, H = 4, 2048, 1024, 16
D = C // H  # 64
NQ = 1024  # query rows per core
IT = C // P  # 8 input-feature tiles
NQT = NQ // P  # 8 query blocks per core
NKT = N // P  # 16 key chunks
NPAIR = 4  # query pairs of 256 rows per core
BF = mybir.dt.bfloat16
F32 = mybir.dt.float32
bf16 = ml_dtypes.bfloat16
EXP = mybir.ActivationFunctionType.Exp
SCALE = float(D) ** -0.5


def _emit_body(nc, pools, dram):
    (consts, acts, xq, wstream, epool, rpool, opool, apsum, spsum, ypsum) = pools
    (qT_d, xT_d, wqT_d, wkT_d, wvT_d, projT_d, bias_d, mask_d, out_d) = dram

    # ---- persistent SBUF tensors ----
    kT_sb = acts.tile([P, IT, N], BF)
    v_sb = acts.tile([P, NKT, H, D + 1], BF)
    qhT_sb = acts.tile([P, IT, NQ], BF)
    yT_sb = qhT_sb  # aliased: each qhT 256-col slice is dead after its pair
    projT_sb = acts.tile([P, IT, C], BF)
    wkT_sb = acts.tile([P, IT, C], BF)
    wvT_sb = acts.tile([P, IT, C], BF)
    mask_sb = consts.tile([P, 4, 256], BF)
    bias_sb = consts.tile([1, C], BF)
    onesb_sb = consts.tile([1, P], BF)

    def load_sliced(dst, dram_ap, engine=None, cols=None):
        """Per-i-tile DMA slices: contiguous DRAM lines + incremental
        availability (consumers depend only on their slice)."""
        eng = engine or nc.sync
        src = dram_ap
        for it in range(IT):
            if cols is None:
                eng.dma_start(dst[:, it], src[:, it])
            else:
                eng.dma_start(dst[:, it], src[:, it, cols])

    # bulk background loads go on the gpsimd (SWDGE) queue so they
    # stream in parallel with the critical-path sync-queue loads
    nc.vector.memset(onesb_sb[:], 1.0)
    nc.vector.memset(v_sb[:, :, :, D : D + 1], 1.0)
    nc.gpsimd.dma_start(wvT_sb[:], wvT_d.rearrange("(i p) o -> p i o", p=P))
    nc.gpsimd.dma_start(mask_sb[:], mask_d.rearrange("(c p) q -> p c q", p=P))
    nc.gpsimd.dma_start(bias_sb[:], bias_d[None, :])
    nc.gpsimd.dma_start(projT_sb[:], projT_d.rearrange("(c p) o -> p c o", p=P))

    # -------- background projection groups --------
    bg = collections.deque()

    def qh_batch(nh):
        qt = xq.tile([P, IT, 512], BF, tag="xq")
        wt = wstream.tile([P, IT, C], BF, tag="wq")
        qsrc = qT_d.rearrange("(i p) n -> p i n", p=P)
        wsrc = wqT_d.rearrange("(i p) o -> p i o", p=P)
        for it in range(IT):  # interleaved so it-slice deps land in order
            nc.sync.dma_start(wt[:, it], wsrc[:, it])
            nc.sync.dma_start(qt[:, it], qsrc[:, it, nh * 512 : (nh + 1) * 512])

        def group(ot):
            def emit():
                ps = apsum.tile([P, 512], F32, tag="acc")
                for it in range(IT):
                    nc.tensor.matmul(
                        ps[:],
                        lhsT=wt[:, it, ot * P : (ot + 1) * P],
                        rhs=qt[:, it, :],
                        start=(it == 0),
                        stop=(it == IT - 1),
                    )
                nc.vector.tensor_copy(qhT_sb[:, ot, nh * 512 : (nh + 1) * 512], ps[:])

            return emit

        return [group(ot) for ot in range(IT)]

    def kv_batch(nh):
        xt = xq.tile([P, IT, 512], BF, tag="xq")
        load_sliced(
            xt, xT_d.rearrange("(i p) n -> p i n", p=P),
            cols=slice(nh * 512, (nh + 1) * 512),
        )

        def kgroup(ot):
            def emit():
                ps = apsum.tile([P, 512], F32, tag="acc")
                for it in range(IT):
                    nc.tensor.matmul(
                        ps[:],
                        lhsT=wkT_sb[:, it, ot * P : (ot + 1) * P],
                        rhs=xt[:, it, :],
                        start=(it == 0),
                        stop=(it == IT - 1),
                    )
                nc.vector.tensor_copy(kT_sb[:, ot, nh * 512 : (nh + 1) * 512], ps[:])

            return emit

        def vgroup(oh, ntl):
            def emit():
                nt = nh * 4 + ntl
                ps = apsum.tile([P, 512], F32, tag="acc")
                for it in range(IT):
                    nc.tensor.matmul(
                        ps[:],
                        lhsT=xt[:, it, ntl * P : (ntl + 1) * P],
                        rhs=wvT_sb[:, it, oh * 512 : (oh + 1) * 512],
                        start=(it == 0),
                        stop=(it == IT - 1),
                    )
                nc.vector.tensor_copy(
                    v_sb[:, nt, oh * 8 : (oh + 1) * 8, 0:D],
                    ps[:].rearrange("p (h d) -> p h d", d=D),
                )

            return emit

        groups = [kgroup(ot) for ot in range(IT)]
        groups += [vgroup(oh, ntl) for oh in range(2) for ntl in range(4)]
        return groups

    def proj_batch(pp):
        def pgroup(nt, oh):
            def emit():
                ps = apsum.tile([P, 512], F32, tag="acc")
                for ct in range(IT):
                    nc.tensor.matmul(
                        ps[:],
                        lhsT=yT_sb[:, ct, nt * P : (nt + 1) * P],
                        rhs=projT_sb[:, ct, oh * 512 : (oh + 1) * 512],
                        start=(ct == 0),
                        stop=False,
                    )
                nc.tensor.matmul(
                    ps[:],
                    lhsT=onesb_sb[0:1, :],
                    rhs=bias_sb[0:1, oh * 512 : (oh + 1) * 512],
                    start=False,
                    stop=True,
                )
                ot_sb = opool.tile([P, 512], F32, tag="o")
                nc.vector.tensor_copy(ot_sb[:], ps[:])
                nc.sync.dma_start(
                    out_d[nt * P : (nt + 1) * P, oh * 512 : (oh + 1) * 512], ot_sb[:]
                )

            return emit

        return [pgroup(nt, oh) for nt in (2 * pp, 2 * pp + 1) for oh in range(2)]

    state = {"steps_left": 1, "credit": 0.0}

    def pop_bg():
        # spread the queued groups over the remaining steps of the
        # CURRENT pair: early pairs have little attention work per step
        # (PE idles while ScalarE runs exp), so drain aggressively
        state["credit"] += len(bg) / max(1, state["steps_left"])
        while state["credit"] >= 1.0 and bg:
            bg.popleft()[1]()
            state["credit"] -= 1.0
        state["steps_left"] = max(1, state["steps_left"] - 1)

    def flush_bg(pair):
        """Emit every queued group some unit of `pair` will read.
        Tile resolves dependencies from emission history only, so a
        writer must always be emitted before its readers."""
        while any(d <= pair for d, _ in bg):
            bg.popleft()[1]()

    # -------- upfront projections (needed by pair 0) --------
    qh0 = qh_batch(0)  # emits qt + wq DMAs first on the sync queue
    load_sliced(wkT_sb, wkT_d.rearrange("(i p) o -> p i o", p=P))
    kv0 = kv_batch(0)
    for g_ in qh0:
        g_()
    for g_ in kv0:
        g_()

    # -------- attention with interleaved background --------
    for p in range(NPAIR):
        if p == 0:
            # qh columns 512.. are first read by pair 2
            bg.extend((2, g) for g in qh_batch(1))
        if p < NPAIR - 1:
            bg.extend((p + 1, g) for g in kv_batch(p + 1))
        if p >= 1:
            bg.extend((NPAIR + 1, g) for g in proj_batch(p - 1))
        flush_bg(p)
        state["steps_left"] = 8 * (p + 2)
        qs = slice(p * 256, (p + 1) * 256)
        for m in range(8):
            h0, h1 = 2 * m, 2 * m + 1
            yps0 = ypsum.tile([P, 256], F32, tag="y")
            yps1 = ypsum.tile([P, 256], F32, tag="y")

            def do_av(ework, last):
                e0, e1, g = ework
                for et, yps, h in ((e0, yps0, h0), (e1, yps1, h1)):
                    for cc in range(4):
                        c = g * 4 + cc
                        nc.tensor.matmul(
                            yps[0 : D + 1, :],
                            lhsT=v_sb[:, c, h, :],
                            rhs=et[:, cc, :],
                            start=(g == 0 and cc == 0),
                            stop=(last and cc == 3),
                        )

            prev = None  # (E0, E1, g)
            for g in range(p + 1):
                pop_bg()
                es = []
                for hb in (0, D):
                    sps = spsum.tile([P, 1024], F32, tag="s")
                    for cc in range(4):
                        c = g * 4 + cc
                        nc.tensor.matmul(
                            sps[:, cc * 256 : (cc + 1) * 256],
                            lhsT=kT_sb[hb : hb + D, m, c * P : (c + 1) * P],
                            rhs=qhT_sb[hb : hb + D, m, qs],
                            start=True,
                            stop=True,
                        )
                    et = epool.tile([P, 4, 256], BF, tag="e")
                    nc.scalar.activation(
                        et[:].rearrange("p a b -> p (a b)"), sps[:], EXP, scale=SCALE
                    )
                    if g == p:
                        nc.vector.tensor_mul(et[:], et[:], mask_sb[:])
                    es.append(et)
                if prev is not None:
                    do_av(prev, last=False)
                prev = (es[0], es[1], g)
            pop_bg()
            do_av(prev, last=True)
            # normalize both heads (fast approx reciprocal: denoms are
            # sums of exps in [1, ~4e3], far from the undefined edges)
            for yps, hb in ((yps0, 0), (yps1, D)):
                dstage = rpool.tile([1, 256], F32, tag="ds")
                nc.vector.tensor_copy(dstage[:], yps[D : D + 1, :])
                r = rpool.tile([1, 256], F32, tag="r")
                nc.vector.reciprocal_approx_fast(r[:], dstage[:])
                rsb = rpool.tile([D, 256], F32, tag="rb")
                nc.gpsimd.partition_broadcast(rsb[:], r[:])
                nc.vector.tensor_mul(yT_sb[hb : hb + D, m, qs], yps[0:D, :], rsb[:])

    bg.extend((NPAIR + 1, g) for g in proj_batch(NPAIR - 1))
    while bg:
        bg.popleft()[1]()


def build_nc(loop_iters=None):
    nc = bacc.Bacc("TRN2", target_bir_lowering=False, debug=False, num_devices=8)

    dram = (
        nc.dram_tensor("qT", [C, NQ], BF, kind="ExternalInput").ap(),
        nc.dram_tensor("xT", [C, N], BF, kind="ExternalInput").ap(),
        nc.dram_tensor("wqT", [C, C], BF, kind="ExternalInput").ap(),
        nc.dram_tensor("wkT", [C, C], BF, kind="ExternalInput").ap(),
        nc.dram_tensor("wvT", [C, C], BF, kind="ExternalInput").ap(),
        nc.dram_tensor("projT", [C, C], BF, kind="ExternalInput").ap(),
        nc.dram_tensor("bias", [C], BF, kind="ExternalInput").ap(),
        nc.dram_tensor("mask", [4 * P, 256], BF, kind="ExternalInput").ap(),
        nc.dram_tensor("out", [NQ, C], F32, kind="ExternalOutput").ap(),
    )

    with tile.TileContext(nc) as tc:
        with (
            tc.tile_pool(name="consts", bufs=1) as consts,
            tc.tile_pool(name="acts", bufs=1) as acts,
            tc.tile_pool(name="xq", bufs=2) as xq,
            tc.tile_pool(name="wstream", bufs=1) as wstream,
            tc.tile_pool(name="epool", bufs=3) as epool,
            tc.tile_pool(name="rpool", bufs=2) as rpool,
            tc.tile_pool(name="opool", bufs=2) as opool,
            tc.tile_pool(name="apsum", bufs=2, space="PSUM") as apsum,
            tc.tile_pool(name="spsum", bufs=2, space="PSUM") as spsum,
            tc.tile_pool(name="ypsum", bufs=2, space="PSUM") as ypsum,
        ):
            pools = (
                consts, acts, xq, wstream, epool, rpool, opool,
                apsum, spsum, ypsum,
            )
            if loop_iters:
                with tc.For_i(0, loop_iters, 1):
                    _emit_body(nc, pools, dram)
            else:
                _emit_body(nc, pools, dram)

    nc.compile()
    return nc


def make_mask(half):
    """Multiplicative causal mask for the 4 tail key chunks vs the 2
    query blocks of a pair: mask[rk, b*128+qq] = rk <= (2b+half)*128+qq."""
    rk = np.arange(4 * P)[:, None]
    b = np.arange(256)[None, :] // P
    qq = np.arange(256)[None, :] % P
    return (rk <= (2 * b + half) * P + qq).astype(bf16)


def prep_inputs(q, x, wq_w, wkv_w, proj_w, proj_b):
    wqT = np.ascontiguousarray(wq_w.T).astype(bf16)
    wkT = np.ascontiguousarray(wkv_w[:C].T).astype(bf16)
    wvT = np.ascontiguousarray(wkv_w[C:].T).astype(bf16)
    projT = np.ascontiguousarray(proj_w.T).astype(bf16)
    bias = proj_b.astype(bf16)
    masks = [make_mask(0), make_mask(1)]
    in_maps = []
    for core in range(8):
        b, half = core // 2, core % 2
        blocks = [2 * j + half for j in range(NQT)]
        qrows = q[b].reshape(NKT, P, C)[blocks].reshape(NQ, C)
        in_maps.append(
            dict(
                qT=np.ascontiguousarray(qrows.T).astype(bf16),
                xT=np.ascontiguousarray(x[b].T).astype(bf16),
                wqT=wqT,
                wkT=wkT,
                wvT=wvT,
                projT=projT,
                bias=bias,
                mask=masks[half],
            )
        )
    return in_maps


def assemble_output(results):
    out = np.empty((B, N, C), np.float32)
    for core in range(8):
        b, half = core // 2, core % 2
        blocks = [2 * j + half for j in range(NQT)]
        out[b].reshape(NKT, P, C)[blocks] = results[core]["out"].reshape(NQT, P, C)
    return out


_CACHE = {}


def kernel(q, x, wq_w, wkv_w, proj_w, proj_b):
    in_maps = prep_inputs(
        np.asarray(q), np.asarray(x), np.asarray(wq_w), np.asarray(wkv_w),
        np.asarray(proj_w), np.asarray(proj_b),
    )
    if "nc" not in _CACHE:
        _CACHE["nc"] = build_nc()
    nc = _CACHE["nc"]
    from concourse.bass_utils import run_bass_kernel_spmd

    res = run_bass_kernel_spmd(nc, in_maps, list(range(8)))
    return assemble_output(res.results)


if __name__ == "__main__":
    rng = np.random.default_rng(0)
    ins = {
        "q": rng.standard_normal((B, N, C)).astype(np.float32),
        "x": rng.standard_normal((B, N, C)).astype(np.float32),
        "wq_w": (rng.standard_normal((C, C)) * 0.02).astype(np.float32),
        "wkv_w": (rng.standard_normal((2 * C, C)) * 0.02).astype(np.float32),
        "proj_w": (rng.standard_normal((C, C)) * 0.02).astype(np.float32),
        "proj_b": np.zeros(C, np.float32),
    }
    out = kernel(**ins)
    print(out.shape, out.dtype)



# revision 2
# speedup vs baseline: 1.0191x; 1.0191x over previous
"""Trainium2 Bass kernel for nn_CrossAttention (B=4, N=2048, C=1024, H=16).

Sharding v2: 8 cores = 4 batches x 2 head-groups of 8 heads. Core
(b, hg) computes q/k/v projections for its 8 heads only (halving the
kv-projection work vs batch x query-half sharding), runs causal
attention for those heads over all 2048 queries of batch b, and emits a
PARTIAL output projection out_hg = y_hg @ projT[hg rows] (K=512) in
bf16. The host sums the two partials per batch and adds the bias during
unshard, so no device collective is needed and the compiled program is
identical on all 8 cores.

Attention processes query pairs jp (256 rows = blocks 2jp, 2jp+1),
attending key chunks 0..2jp+1 in groups of 2 chunks x 2 head-halves:
  S^T[k,q] = kT^T qhT per (chunk, hb) into one [128, 4, 256] psum
  E = exp(0.125 S^T) (single [128,1024] activation per group)
  tail group (g==jp) gets a static multiplicative causal mask
  [Y^T; denom] accumulate via lhsT=[v|1] chunk matmuls, per head
  yT = Y^T * bcast(1/denom)
Projection/bg groups (k/v/qh for later pairs, partial out-proj) are
interleaved into the attention stream with deadline-aware pacing so the
PE stays busy while ScalarE computes exponentials. ~40 warmup matmuls
run during the initial DMA window to lift the HAM clock gate to 8/8
before real work lands.
"""

import collections

import numpy as np
import ml_dtypes

import concourse.bass as bass
import concourse.tile as tile
from concourse import bacc, mybir

P = 128
B, N, C, H = 4, 2048, 1024, 16
D = C // H  # 64
HG = 8  # heads per core
M4 = HG // 2  # head pairs per core
IT = C // P  # 8 input-feature tiles
NKT = N // P  # 16 key chunks
NJP = N // 256  # 8 query pair-blocks per core
BF = mybir.dt.bfloat16
F32 = mybir.dt.float32
bf16 = ml_dtypes.bfloat16
EXP = mybir.ActivationFunctionType.Exp
SCALE = float(D) ** -0.5
NWARM = 48


def _emit_body(nc, pools, dram):
    (consts, acts, qpool, xpool, epool, rpool, opool, apsum, spsum, ypsum) = pools
    (qT_d, xT_d, wqT_d, wkT_d, wvT_d, projT_d, mask_d, out_d) = dram

    # ---- persistent SBUF tensors ----
    kT_sb = acts.tile([P, M4, N], BF)
    qhT_sb = acts.tile([P, M4, N], BF)
    yT_sb = qhT_sb  # aliased: qhT (m, jp) slice dead after pair jp scores
    v_sb = acts.tile([P, NKT, HG, D + 1], BF)
    projT_sb = acts.tile([P, M4, C], BF)
    wq_sb = consts.tile([P, IT, 512], BF)
    wk_sb = consts.tile([P, IT, 512], BF)
    wv_sb = consts.tile([P, IT, 512], BF)
    mask_sb = consts.tile([P, 512], BF)
    warm_sb = consts.tile([P, 512], BF)

    nc.vector.memset(warm_sb[:], 0.01)
    nc.vector.memset(v_sb[:, :, :, D : D + 1], 1.0)

    # ---- HAM warmup: keep PE busy during the initial DMA window ----
    if NWARM:
        wps = ypsum.tile([P, 256], F32, tag="y")
        for _ in range(NWARM):
            nc.tensor.matmul(
                wps[:], lhsT=warm_sb[:, 0:128], rhs=warm_sb[:, 0:256],
                start=True, stop=True,
            )

    # ---- phase-0 DMA: finely interleaved, critical-first ----
    qsrc = qT_d.rearrange("(i p) n -> p i n", p=P)
    xsrc = xT_d.rearrange("(i p) n -> p i n", p=P)
    xt0 = xpool.tile([P, IT, 512], BF, tag="xt")
    qt0 = qpool.tile([P, IT, 512], BF, tag="qt")
    for it in range(IT):
        nc.sync.dma_start(wk_sb[:, it], wkT_d.rearrange("(i p) o -> p i o", p=P)[:, it])
        nc.sync.dma_start(xt0[:, it], xsrc[:, it, 0:512])
        nc.sync.dma_start(wv_sb[:, it], wvT_d.rearrange("(i p) o -> p i o", p=P)[:, it])
        nc.sync.dma_start(wq_sb[:, it], wqT_d.rearrange("(i p) o -> p i o", p=P)[:, it])
        nc.sync.dma_start(qt0[:, it], qsrc[:, it, 0:512])
    # bulk loads not needed until later: software-DGE queue
    nc.gpsimd.dma_start(projT_sb[:], projT_d.rearrange("(m p) o -> p m o", p=P))
    nc.gpsimd.dma_start(mask_sb[:].rearrange("p (s q) -> p s q", q=256), mask_d.rearrange("(s p) q -> p s q", p=P))
    # prefetch the nh=1 streams so jp0's background groups never wait on DMA
    xt1 = xpool.tile([P, IT, 512], BF, tag="xt")
    qt1 = qpool.tile([P, IT, 512], BF, tag="qt")
    for it in range(IT):
        nc.sync.dma_start(xt1[:, it], xsrc[:, it, 512:1024])
        nc.sync.dma_start(qt1[:, it], qsrc[:, it, 512:1024])

    # -------- projection group constructors --------
    def acc_ps(pool):
        """Accumulation psum: apsum ring, or first half of an spsum tile
        (same tag/ring as attention scores, which are idle in phase 0)."""
        if pool is None:
            pst = apsum.tile([P, 512], F32, tag="acc")
            return pst[:]
        pst = pool.tile([P, 1024], F32, tag="s")
        return pst[:, 0:512]

    def k_group(m, nh, xt, pool=None):
        def emit():
            ps = acc_ps(pool)
            for it in range(IT):
                nc.tensor.matmul(
                    ps, lhsT=wk_sb[:, it, m * P : (m + 1) * P], rhs=xt[:, it, :],
                    start=(it == 0), stop=(it == IT - 1),
                )
            nc.vector.tensor_copy(kT_sb[:, m, nh * 512 : (nh + 1) * 512], ps)
        return emit

    def qh_group(m, nh, qt, pool=None):
        def emit():
            ps = acc_ps(pool)
            for it in range(IT):
                nc.tensor.matmul(
                    ps, lhsT=wq_sb[:, it, m * P : (m + 1) * P], rhs=qt[:, it, :],
                    start=(it == 0), stop=(it == IT - 1),
                )
            nc.vector.tensor_copy(qhT_sb[:, m, nh * 512 : (nh + 1) * 512], ps)
        return emit

    def v_group(c, xt, pool=None):
        def emit():
            ps = acc_ps(pool)
            cl = c % 4
            for it in range(IT):
                nc.tensor.matmul(
                    ps, lhsT=xt[:, it, cl * P : (cl + 1) * P], rhs=wv_sb[:, it, :],
                    start=(it == 0), stop=(it == IT - 1),
                )
            nc.vector.tensor_copy(
                v_sb[:, c, :, 0:D], ps.rearrange("p (h d) -> p h d", d=D)
            )
        return emit

    def proj_group(j, oh):
        def emit():
            ps = apsum.tile([P, 512], F32, tag="acc")
            for m in range(M4):
                nc.tensor.matmul(
                    ps[:],
                    lhsT=yT_sb[:, m, j * P : (j + 1) * P],
                    rhs=projT_sb[:, m, oh * 512 : (oh + 1) * 512],
                    start=(m == 0), stop=(m == M4 - 1),
                )
            ot = opool.tile([P, 512], BF, tag="o")
            nc.vector.tensor_copy(ot[:], ps[:])
            nc.sync.dma_start(
                out_d[j * P : (j + 1) * P, oh * 512 : (oh + 1) * 512], ot[:]
            )
        return emit

    def kv_batch(nh, xt=None):
        """k(m, nh) x4 + v(4nh..4nh+3); loads its xt slice unless given."""
        if xt is None:
            xt = xpool.tile([P, IT, 512], BF, tag="xt")
            for it in range(IT):
                nc.sync.dma_start(xt[:, it], xsrc[:, it, nh * 512 : (nh + 1) * 512])
        out = [(2 * nh, k_group(m, nh, xt)) for m in range(M4)]
        out += [((4 * nh + cl - 1) // 2, v_group(4 * nh + cl, xt)) for cl in range(4)]
        return out

    def qh_batch(nh, qt=None):
        if qt is None:
            qt = qpool.tile([P, IT, 512], BF, tag="qt")
            for it in range(IT):
                nc.sync.dma_start(qt[:, it], qsrc[:, it, nh * 512 : (nh + 1) * 512])
        return [(2 * nh, qh_group(m, nh, qt)) for m in range(M4)]

    # -------- phase 0: projections needed by jp 0-1 --------
    # phase-0 groups, emission order matches DMA arrival (k/v before qh);
    # alternate psum pools (spsum is idle until attention) for 4-wide
    # accumulation during the DMA-bound ramp
    g0 = []
    for m in range(M4):
        g0.append(k_group(m, 0, xt0, pool=(spsum if m % 2 else None)))
    v0 = [v_group(c, xt0, pool=(spsum if c % 2 else None)) for c in range(4)]
    q0 = []
    for m in range(M4):
        q0.append(qh_group(m, 0, qt0, pool=(spsum if m % 2 else None)))
    for g in (g0[0], v0[0], v0[1], q0[0], g0[1], v0[2], v0[3], q0[1],
              g0[2], q0[2], g0[3], q0[3]):
        g()

    # -------- background queue with deadline pacing --------
    bg = collections.deque()
    state = {"jp": 0, "step": 0, "credit": 0.0}

    def steps_till(dead):
        d = min(dead, NJP)
        s = 4 * (state["jp"] // 2 + 1) - state["step"]
        for j in range(state["jp"] + 1, d):
            s += 4 * (j // 2 + 1)
        return max(1, s)

    def pop_bg():
        if bg:
            dead = min(d for d, _ in bg)
            state["credit"] += len(bg) / steps_till(dead)
            while state["credit"] >= 1.0 and bg:
                bg.popleft()[1]()
                state["credit"] -= 1.0
        state["step"] += 1

    def flush_bg(jp):
        while any(d <= jp for d, _ in bg):
            bg.popleft()[1]()

    # -------- attention --------
    for jp in range(NJP):
        if jp == 0:
            bg.extend(kv_batch(1, xt1))
            bg.extend(qh_batch(1, qt1))
        elif jp == 1:
            bg.extend(kv_batch(2))
            bg.extend(qh_batch(2))
        elif jp == 2:
            bg.extend(kv_batch(3))
            bg.extend(qh_batch(3))
        flush_bg(jp)
        state["jp"] = jp
        state["step"] = 0
        qs = slice(jp * 256, (jp + 1) * 256)
        nch = 2 * jp + 2
        gs = [(c0, min(4, nch - c0)) for c0 in range(0, nch, 4)]
        for m in range(M4):
            yps0 = ypsum.tile([P, 256], F32, tag="y")
            yps1 = ypsum.tile([P, 256], F32, tag="y")
            yps01 = (yps0, yps1)

            def do_av(ework, last):
                es, c0, w = ework
                for hbix in (0, 1):
                    et = es[hbix]
                    for cc in range(w):
                        c = c0 + cc
                        nc.tensor.matmul(
                            yps01[hbix][0 : D + 1, :],
                            lhsT=v_sb[:, c, 2 * m + hbix, :],
                            rhs=et[:, cc * 256 : (cc + 1) * 256],
                            start=(c == 0),
                            stop=(last and cc == w - 1),
                        )

            prev = None
            for c0, w in gs:
                pop_bg()
                es = []
                for hbix, hb in ((0, 0), (1, D)):
                    sps = spsum.tile([P, 1024], F32, tag="s")
                    for cc in range(w):
                        c = c0 + cc
                        nc.tensor.matmul(
                            sps[:, cc * 256 : (cc + 1) * 256],
                            lhsT=kT_sb[hb : hb + D, m, c * P : (c + 1) * P],
                            rhs=qhT_sb[hb : hb + D, m, qs],
                            start=True, stop=True,
                        )
                    et = epool.tile([P, 1024], BF, tag="e")
                    nc.scalar.activation(
                        et[:, : w * 256], sps[:, : w * 256], EXP, scale=SCALE
                    )
                    if c0 + w == nch:  # tail group: mask last 2 chunks
                        nc.vector.tensor_mul(
                            et[:, (w - 2) * 256 : w * 256],
                            et[:, (w - 2) * 256 : w * 256],
                            mask_sb[:],
                        )
                    es.append(et)
                if prev is not None:
                    do_av(prev, last=False)
                prev = (es, c0, w)
            pop_bg()
            do_av(prev, last=True)
            # normalize both heads (denoms in [1, ~4e3], approx recip ok)
            for hbix, hb in ((0, 0), (1, D)):
                yps = yps01[hbix]
                dstage = rpool.tile([1, 256], F32, tag="ds")
                nc.vector.tensor_copy(dstage[:], yps[D : D + 1, :])
                r = rpool.tile([1, 256], F32, tag="r")
                nc.vector.reciprocal_approx_fast(r[:], dstage[:])
                rsb = rpool.tile([D, 256], F32, tag="rb")
                nc.gpsimd.partition_broadcast(rsb[:], r[:])
                nc.vector.tensor_mul(
                    yT_sb[hb : hb + D, m, qs], yps[0:D, :], rsb[:]
                )
        for j in (2 * jp, 2 * jp + 1):
            for oh in range(2):
                bg.append((min(jp + 2, NJP + 1), proj_group(j, oh)))

    while bg:
        bg.popleft()[1]()


def build_nc(loop_iters=None):
    nc = bacc.Bacc("TRN2", target_bir_lowering=False, debug=False, num_devices=8)

    dram = (
        nc.dram_tensor("qT", [C, N], BF, kind="ExternalInput").ap(),
        nc.dram_tensor("xT", [C, N], BF, kind="ExternalInput").ap(),
        nc.dram_tensor("wqT", [C, 512], BF, kind="ExternalInput").ap(),
        nc.dram_tensor("wkT", [C, 512], BF, kind="ExternalInput").ap(),
        nc.dram_tensor("wvT", [C, 512], BF, kind="ExternalInput").ap(),
        nc.dram_tensor("projT", [512, C], BF, kind="ExternalInput").ap(),
        nc.dram_tensor("mask", [2 * P, 256], BF, kind="ExternalInput").ap(),
        nc.dram_tensor("out", [N, C], BF, kind="ExternalOutput").ap(),
    )

    with tile.TileContext(nc) as tc:
        with (
            tc.tile_pool(name="consts", bufs=1) as consts,
            tc.tile_pool(name="acts", bufs=1) as acts,
            tc.tile_pool(name="qpool", bufs=2) as qpool,
            tc.tile_pool(name="xpool", bufs=2) as xpool,
            tc.tile_pool(name="epool", bufs=6) as epool,
            tc.tile_pool(name="rpool", bufs=2) as rpool,
            tc.tile_pool(name="opool", bufs=2) as opool,
            tc.tile_pool(name="apsum", bufs=2, space="PSUM") as apsum,
            tc.tile_pool(name="spsum", bufs=2, space="PSUM") as spsum,
            tc.tile_pool(name="ypsum", bufs=2, space="PSUM") as ypsum,
        ):
            pools = (
                consts, acts, qpool, xpool, epool, rpool, opool,
                apsum, spsum, ypsum,
            )
            if loop_iters:
                with tc.For_i(0, loop_iters, 1):
                    _emit_body(nc, pools, dram)
            else:
                _emit_body(nc, pools, dram)

    nc.compile()
    return nc


def make_mask():
    """Static tail mask [4*128, 256]: slots (c_t0 hb0, c_t0 hb1, c_t1 hb0,
    c_t1 hb1) for the last two key chunks of pair jp vs its two query
    blocks (delta = q block within pair). Chunk 2jp (t0): triangular for
    delta=0, visible for delta=1. Chunk 2jp+1 (t1): invisible for
    delta=0, triangular for delta=1."""
    k = np.arange(P)[:, None]
    dlt = np.arange(256)[None, :] // P
    qq = np.arange(256)[None, :] % P
    a = ((dlt == 1) | (k <= qq)).astype(bf16)
    b_ = ((dlt == 1) & (k <= qq)).astype(bf16)
    return np.concatenate([a, b_], axis=0)


def prep_inputs(q, x, wq_w, wkv_w, proj_w, proj_b):
    projT_full = np.ascontiguousarray(proj_w.T).astype(bf16)
    mask = make_mask()
    in_maps = []
    for core in range(8):
        b, hg = core // 2, core % 2
        hs = slice(hg * 512, (hg + 1) * 512)
        in_maps.append(
            dict(
                qT=np.ascontiguousarray(q[b].T).astype(bf16),
                xT=np.ascontiguousarray(x[b].T).astype(bf16),
                wqT=np.ascontiguousarray(wq_w[hs].T).astype(bf16),
                wkT=np.ascontiguousarray(wkv_w[:C][hs].T).astype(bf16),
                wvT=np.ascontiguousarray(wkv_w[C:][hs].T).astype(bf16),
                projT=np.ascontiguousarray(projT_full[hs]),
                mask=mask,
            )
        )
    return in_maps


def assemble_output(results, proj_b):
    out = np.empty((B, N, C), np.float32)
    for b in range(B):
        out[b] = (
            results[2 * b]["out"].astype(np.float32)
            + results[2 * b + 1]["out"].astype(np.float32)
            + proj_b[None, :].astype(np.float32)
        )
    return out


_CACHE = {}


def kernel(q, x, wq_w, wkv_w, proj_w, proj_b):
    proj_b = np.asarray(proj_b)
    in_maps = prep_inputs(
        np.asarray(q), np.asarray(x), np.asarray(wq_w), np.asarray(wkv_w),
        np.asarray(proj_w), proj_b,
    )
    if "nc" not in _CACHE:
        _CACHE["nc"] = build_nc()
    nc = _CACHE["nc"]
    from concourse.bass_utils import run_bass_kernel_spmd

    res = run_bass_kernel_spmd(nc, in_maps, list(range(8)))
    return assemble_output(res.results, proj_b)


if __name__ == "__main__":
    rng = np.random.default_rng(0)
    ins = {
        "q": rng.standard_normal((B, N, C)).astype(np.float32),
        "x": rng.standard_normal((B, N, C)).astype(np.float32),
        "wq_w": (rng.standard_normal((C, C)) * 0.02).astype(np.float32),
        "wkv_w": (rng.standard_normal((2 * C, C)) * 0.02).astype(np.float32),
        "proj_w": (rng.standard_normal((C, C)) * 0.02).astype(np.float32),
        "proj_b": np.zeros(C, np.float32),
    }
    out = kernel(**ins)
    print(out.shape, out.dtype)


# revision 3
# speedup vs baseline: 1.0476x; 1.0280x over previous
"""Trainium2 Bass kernel for nn_CrossAttention (B=4, N=2048, C=1024, H=16).

Sharding v2: 8 cores = 4 batches x 2 head-groups of 8 heads. Core
(b, hg) computes q/k/v projections for its 8 heads only (halving the
kv-projection work vs batch x query-half sharding), runs causal
attention for those heads over all 2048 queries of batch b, and emits a
PARTIAL output projection out_hg = y_hg @ projT[hg rows] (K=512) in
bf16. The host sums the two partials per batch and adds the bias during
unshard, so no device collective is needed and the compiled program is
identical on all 8 cores.

Attention processes query pairs jp (256 rows = blocks 2jp, 2jp+1),
attending key chunks 0..2jp+1 in groups of 2 chunks x 2 head-halves:
  S^T[k,q] = kT^T qhT per (chunk, hb) into one [128, 4, 256] psum
  E = exp(0.125 S^T) (single [128,1024] activation per group)
  tail group (g==jp) gets a static multiplicative causal mask
  [Y^T; denom] accumulate via lhsT=[v|1] chunk matmuls, per head
  yT = Y^T * bcast(1/denom)
Projection/bg groups (k/v/qh for later pairs, partial out-proj) are
interleaved into the attention stream with deadline-aware pacing so the
PE stays busy while ScalarE computes exponentials. ~40 warmup matmuls
run during the initial DMA window to lift the HAM clock gate to 8/8
before real work lands.
"""

import collections

import numpy as np
import ml_dtypes

import concourse.bass as bass
import concourse.tile as tile
from concourse import bacc, mybir

P = 128
B, N, C, H = 4, 2048, 1024, 16
D = C // H  # 64
HG = 8  # heads per core
M4 = HG // 2  # head pairs per core
IT = C // P  # 8 input-feature tiles
NKT = N // P  # 16 key chunks
NJP = N // 256  # 8 query pair-blocks per core
BF = mybir.dt.bfloat16
F32 = mybir.dt.float32
bf16 = ml_dtypes.bfloat16
EXP = mybir.ActivationFunctionType.Exp
SCALE = float(D) ** -0.5
NWARM = 48


def _emit_body(nc, pools, dram):
    (consts, acts, qpool, xpool, epool, rpool, opool, apsum, spsum, ypsum) = pools
    (qT_d, xT_d, wqT_d, wkT_d, wvT_d, projT_d, mask_d, out_d) = dram

    # ---- persistent SBUF tensors ----
    kT_sb = acts.tile([P, M4, N], BF)
    qhT_sb = acts.tile([P, M4, N], BF)
    yT_sb = qhT_sb  # aliased: qhT (m, jp) slice dead after pair jp scores
    v_sb = acts.tile([P, NKT, HG, D + 1], BF)
    projT_sb = acts.tile([P, M4, C], BF)
    wq_sb = consts.tile([P, IT, 512], BF)
    wk_sb = consts.tile([P, IT, 512], BF)
    wv_sb = consts.tile([P, IT, 512], BF)
    mask_sb = consts.tile([P, 512], BF)
    warm_sb = consts.tile([P, 512], BF)

    nc.vector.memset(warm_sb[:], 0.01)
    nc.vector.memset(v_sb[:, :, :, D : D + 1], 1.0)

    # ---- HAM warmup: keep PE busy during the initial DMA window ----
    if NWARM:
        wps = ypsum.tile([P, 256], F32, tag="y")
        for _ in range(NWARM):
            nc.tensor.matmul(
                wps[:], lhsT=warm_sb[:, 0:128], rhs=warm_sb[:, 0:256],
                start=True, stop=True,
            )

    # ---- phase-0 DMA: finely interleaved, critical-first ----
    qsrc = qT_d.rearrange("(i p) n -> p i n", p=P)
    xsrc = xT_d.rearrange("(i p) n -> p i n", p=P)
    xt0 = xpool.tile([P, IT, 512], BF, tag="xt")
    qt0 = qpool.tile([P, IT, 512], BF, tag="qt")
    for it in range(IT):
        nc.sync.dma_start(wk_sb[:, it], wkT_d.rearrange("(i p) o -> p i o", p=P)[:, it])
        nc.sync.dma_start(xt0[:, it], xsrc[:, it, 0:512])
        nc.sync.dma_start(wv_sb[:, it], wvT_d.rearrange("(i p) o -> p i o", p=P)[:, it])
        nc.sync.dma_start(wq_sb[:, it], wqT_d.rearrange("(i p) o -> p i o", p=P)[:, it])
        nc.sync.dma_start(qt0[:, it], qsrc[:, it, 0:512])
    # bulk loads not needed until later: software-DGE queue
    nc.gpsimd.dma_start(projT_sb[:], projT_d.rearrange("(m p) o -> p m o", p=P))
    nc.gpsimd.dma_start(mask_sb[:].rearrange("p (s q) -> p s q", q=256), mask_d.rearrange("(s p) q -> p s q", p=P))
    # prefetch the nh=1 streams so jp0's background groups never wait on DMA
    xt1 = xpool.tile([P, IT, 512], BF, tag="xt")
    qt1 = qpool.tile([P, IT, 512], BF, tag="qt")
    for it in range(IT):
        nc.sync.dma_start(xt1[:, it], xsrc[:, it, 512:1024])
        nc.sync.dma_start(qt1[:, it], qsrc[:, it, 512:1024])

    # -------- projection group constructors --------
    def acc_ps(pool):
        """Accumulation psum: apsum ring, or first half of an spsum tile
        (same tag/ring as attention scores, which are idle in phase 0)."""
        if pool is None:
            pst = apsum.tile([P, 512], F32, tag="acc")
            return pst[:]
        pst = pool.tile([P, 1024], F32, tag="s")
        return pst[:, 0:512]

    def k_group(m, nh, xt, pool=None):
        def emit():
            ps = acc_ps(pool)
            for it in range(IT):
                nc.tensor.matmul(
                    ps, lhsT=wk_sb[:, it, m * P : (m + 1) * P], rhs=xt[:, it, :],
                    start=(it == 0), stop=(it == IT - 1),
                )
            nc.vector.tensor_copy(kT_sb[:, m, nh * 512 : (nh + 1) * 512], ps)
        return emit

    def qh_group(m, nh, qt, pool=None):
        def emit():
            ps = acc_ps(pool)
            for it in range(IT):
                nc.tensor.matmul(
                    ps, lhsT=wq_sb[:, it, m * P : (m + 1) * P], rhs=qt[:, it, :],
                    start=(it == 0), stop=(it == IT - 1),
                )
            nc.vector.tensor_copy(qhT_sb[:, m, nh * 512 : (nh + 1) * 512], ps)
        return emit

    def v_group(c, xt, pool=None):
        def emit():
            ps = acc_ps(pool)
            cl = c % 4
            for it in range(IT):
                nc.tensor.matmul(
                    ps, lhsT=xt[:, it, cl * P : (cl + 1) * P], rhs=wv_sb[:, it, :],
                    start=(it == 0), stop=(it == IT - 1),
                )
            nc.vector.tensor_copy(
                v_sb[:, c, :, 0:D], ps.rearrange("p (h d) -> p h d", d=D)
            )
        return emit

    def proj_group(j, oh):
        def emit():
            ps = apsum.tile([P, 512], F32, tag="acc")
            for m in range(M4):
                nc.tensor.matmul(
                    ps[:],
                    lhsT=yT_sb[:, m, j * P : (j + 1) * P],
                    rhs=projT_sb[:, m, oh * 512 : (oh + 1) * 512],
                    start=(m == 0), stop=(m == M4 - 1),
                )
            ot = opool.tile([P, 512], BF, tag="o")
            nc.vector.tensor_copy(ot[:], ps[:])
            nc.sync.dma_start(
                out_d[j * P : (j + 1) * P, oh * 512 : (oh + 1) * 512], ot[:]
            )
        return emit

    def kv_batch(nh, xt=None):
        """k(m, nh) x4 + v(4nh..4nh+3); loads its xt slice unless given."""
        if xt is None:
            xt = xpool.tile([P, IT, 512], BF, tag="xt")
            for it in range(IT):
                nc.sync.dma_start(xt[:, it], xsrc[:, it, nh * 512 : (nh + 1) * 512])
        out = [(2 * nh, k_group(m, nh, xt)) for m in range(M4)]
        out += [((4 * nh + cl - 1) // 2, v_group(4 * nh + cl, xt)) for cl in range(4)]
        return out

    def qh_batch(nh, qt=None):
        if qt is None:
            qt = qpool.tile([P, IT, 512], BF, tag="qt")
            for it in range(IT):
                nc.sync.dma_start(qt[:, it], qsrc[:, it, nh * 512 : (nh + 1) * 512])
        return [(2 * nh, qh_group(m, nh, qt)) for m in range(M4)]

    # -------- phase 0: projections needed by jp 0-1 --------
    # phase-0 groups, emission order matches DMA arrival (k/v before qh);
    # alternate psum pools (spsum is idle until attention) for 4-wide
    # accumulation during the DMA-bound ramp
    g0 = []
    for m in range(M4):
        g0.append(k_group(m, 0, xt0, pool=(spsum if m % 2 else None)))
    v0 = [v_group(c, xt0, pool=(spsum if c % 2 else None)) for c in range(4)]
    q0 = []
    for m in range(M4):
        q0.append(qh_group(m, 0, qt0, pool=(spsum if m % 2 else None)))
    wfill = ypsum.tile([P, 256], F32, tag="y")
    for g in (g0[0], v0[0], v0[1], q0[0], g0[1], v0[2], v0[3], q0[1],
              g0[2], q0[2], g0[3], q0[3]):
        g()
        for _ in range(4):
            nc.tensor.matmul(
                wfill[:], lhsT=warm_sb[:, 0:128], rhs=warm_sb[:, 0:256],
                start=True, stop=True,
            )

    # -------- background queue with deadline pacing --------
    bg = collections.deque()
    state = {"jp": 0, "step": 0, "credit": 0.0}

    def steps_till(dead):
        d = min(dead, NJP)
        s = 4 * (state["jp"] // 2 + 1) - state["step"]
        for j in range(state["jp"] + 1, d):
            s += 4 * (j // 2 + 1)
        return max(1, s)

    def pop_bg():
        if bg:
            dead = min(d for d, _ in bg)
            state["credit"] += len(bg) / steps_till(dead)
            while state["credit"] >= 1.0 and bg:
                bg.popleft()[1]()
                state["credit"] -= 1.0
        state["step"] += 1

    def flush_bg(jp):
        while any(d <= jp for d, _ in bg):
            bg.popleft()[1]()

    # -------- attention --------
    for jp in range(NJP):
        if jp == 0:
            bg.extend(kv_batch(1, xt1))
            bg.extend(qh_batch(1, qt1))
        elif jp == 1:
            bg.extend(kv_batch(2))
            bg.extend(qh_batch(2))
        elif jp == 2:
            bg.extend(kv_batch(3))
            bg.extend(qh_batch(3))
        flush_bg(jp)
        state["jp"] = jp
        state["step"] = 0
        qs = slice(jp * 256, (jp + 1) * 256)
        nch = 2 * jp + 2
        gs = [(c0, min(4, nch - c0)) for c0 in range(0, nch, 4)]
        for m in range(M4):
            yps0 = ypsum.tile([P, 256], F32, tag="y")
            yps1 = ypsum.tile([P, 256], F32, tag="y")
            yps01 = (yps0, yps1)

            def do_av(ework, last):
                es, c0, w = ework
                for hbix in (0, 1):
                    et = es[hbix]
                    for cc in range(w):
                        c = c0 + cc
                        nc.tensor.matmul(
                            yps01[hbix][0 : D + 1, :],
                            lhsT=v_sb[:, c, 2 * m + hbix, :],
                            rhs=et[:, cc * 256 : (cc + 1) * 256],
                            start=(c == 0),
                            stop=(last and cc == w - 1),
                        )

            prev = None
            for c0, w in gs:
                pop_bg()
                es = []
                for hbix, hb in ((0, 0), (1, D)):
                    sps = spsum.tile([P, 1024], F32, tag="s")
                    for cc in range(w):
                        c = c0 + cc
                        nc.tensor.matmul(
                            sps[:, cc * 256 : (cc + 1) * 256],
                            lhsT=kT_sb[hb : hb + D, m, c * P : (c + 1) * P],
                            rhs=qhT_sb[hb : hb + D, m, qs],
                            start=True, stop=True,
                        )
                    et = epool.tile([P, 1024], BF, tag="e")
                    nc.scalar.activation(
                        et[:, : w * 256], sps[:, : w * 256], EXP, scale=SCALE
                    )
                    if c0 + w == nch:  # tail group: mask last 2 chunks
                        nc.vector.tensor_mul(
                            et[:, (w - 2) * 256 : w * 256],
                            et[:, (w - 2) * 256 : w * 256],
                            mask_sb[:],
                        )
                    es.append(et)
                if prev is not None:
                    do_av(prev, last=False)
                prev = (es, c0, w)
            pop_bg()
            do_av(prev, last=True)
            # normalize both heads (denoms in [1, ~4e3], approx recip ok)
            for hbix, hb in ((0, 0), (1, D)):
                yps = yps01[hbix]
                dstage = rpool.tile([1, 256], F32, tag="ds")
                nc.vector.tensor_copy(dstage[:], yps[D : D + 1, :])
                r = rpool.tile([1, 256], F32, tag="r")
                nc.vector.reciprocal_approx_fast(r[:], dstage[:])
                rsb = rpool.tile([D, 256], F32, tag="rb")
                nc.gpsimd.partition_broadcast(rsb[:], r[:])
                nc.vector.tensor_mul(
                    yT_sb[hb : hb + D, m, qs], yps[0:D, :], rsb[:]
                )
        for j in (2 * jp, 2 * jp + 1):
            for oh in range(2):
                bg.append((min(jp + 2, NJP + 1), proj_group(j, oh)))

    while bg:
        bg.popleft()[1]()


def build_nc(loop_iters=None):
    nc = bacc.Bacc("TRN2", target_bir_lowering=False, debug=False, num_devices=8)

    dram = (
        nc.dram_tensor("qT", [C, N], BF, kind="ExternalInput").ap(),
        nc.dram_tensor("xT", [C, N], BF, kind="ExternalInput").ap(),
        nc.dram_tensor("wqT", [C, 512], BF, kind="ExternalInput").ap(),
        nc.dram_tensor("wkT", [C, 512], BF, kind="ExternalInput").ap(),
        nc.dram_tensor("wvT", [C, 512], BF, kind="ExternalInput").ap(),
        nc.dram_tensor("projT", [512, C], BF, kind="ExternalInput").ap(),
        nc.dram_tensor("mask", [2 * P, 256], BF, kind="ExternalInput").ap(),
        nc.dram_tensor("out", [N, C], BF, kind="ExternalOutput").ap(),
    )

    with tile.TileContext(nc) as tc:
        with (
            tc.tile_pool(name="consts", bufs=1) as consts,
            tc.tile_pool(name="acts", bufs=1) as acts,
            tc.tile_pool(name="qpool", bufs=2) as qpool,
            tc.tile_pool(name="xpool", bufs=2) as xpool,
            tc.tile_pool(name="epool", bufs=6) as epool,
            tc.tile_pool(name="rpool", bufs=2) as rpool,
            tc.tile_pool(name="opool", bufs=2) as opool,
            tc.tile_pool(name="apsum", bufs=2, space="PSUM") as apsum,
            tc.tile_pool(name="spsum", bufs=2, space="PSUM") as spsum,
            tc.tile_pool(name="ypsum", bufs=2, space="PSUM") as ypsum,
        ):
            pools = (
                consts, acts, qpool, xpool, epool, rpool, opool,
                apsum, spsum, ypsum,
            )
            if loop_iters:
                with tc.For_i(0, loop_iters, 1):
                    _emit_body(nc, pools, dram)
            else:
                _emit_body(nc, pools, dram)

    nc.compile()
    return nc


def make_mask():
    """Static tail mask [4*128, 256]: slots (c_t0 hb0, c_t0 hb1, c_t1 hb0,
    c_t1 hb1) for the last two key chunks of pair jp vs its two query
    blocks (delta = q block within pair). Chunk 2jp (t0): triangular for
    delta=0, visible for delta=1. Chunk 2jp+1 (t1): invisible for
    delta=0, triangular for delta=1."""
    k = np.arange(P)[:, None]
    dlt = np.arange(256)[None, :] // P
    qq = np.arange(256)[None, :] % P
    a = ((dlt == 1) | (k <= qq)).astype(bf16)
    b_ = ((dlt == 1) & (k <= qq)).astype(bf16)
    return np.concatenate([a, b_], axis=0)


def prep_inputs(q, x, wq_w, wkv_w, proj_w, proj_b):
    projT_full = np.ascontiguousarray(proj_w.T).astype(bf16)
    mask = make_mask()
    in_maps = []
    for core in range(8):
        b, hg = core // 2, core % 2
        hs = slice(hg * 512, (hg + 1) * 512)
        in_maps.append(
            dict(
                qT=np.ascontiguousarray(q[b].T).astype(bf16),
                xT=np.ascontiguousarray(x[b].T).astype(bf16),
                wqT=np.ascontiguousarray(wq_w[hs].T).astype(bf16),
                wkT=np.ascontiguousarray(wkv_w[:C][hs].T).astype(bf16),
                wvT=np.ascontiguousarray(wkv_w[C:][hs].T).astype(bf16),
                projT=np.ascontiguousarray(projT_full[hs]),
                mask=mask,
            )
        )
    return in_maps


def assemble_output(results, proj_b):
    out = np.empty((B, N, C), np.float32)
    for b in range(B):
        out[b] = (
            results[2 * b]["out"].astype(np.float32)
            + results[2 * b + 1]["out"].astype(np.float32)
            + proj_b[None, :].astype(np.float32)
        )
    return out


_CACHE = {}


def kernel(q, x, wq_w, wkv_w, proj_w, proj_b):
    proj_b = np.asarray(proj_b)
    in_maps = prep_inputs(
        np.asarray(q), np.asarray(x), np.asarray(wq_w), np.asarray(wkv_w),
        np.asarray(proj_w), proj_b,
    )
    if "nc" not in _CACHE:
        _CACHE["nc"] = build_nc()
    nc = _CACHE["nc"]
    from concourse.bass_utils import run_bass_kernel_spmd

    res = run_bass_kernel_spmd(nc, in_maps, list(range(8)))
    return assemble_output(res.results, proj_b)


if __name__ == "__main__":
    rng = np.random.default_rng(0)
    ins = {
        "q": rng.standard_normal((B, N, C)).astype(np.float32),
        "x": rng.standard_normal((B, N, C)).astype(np.float32),
        "wq_w": (rng.standard_normal((C, C)) * 0.02).astype(np.float32),
        "wkv_w": (rng.standard_normal((2 * C, C)) * 0.02).astype(np.float32),
        "proj_w": (rng.standard_normal((C, C)) * 0.02).astype(np.float32),
        "proj_b": np.zeros(C, np.float32),
    }
    out = kernel(**ins)
    print(out.shape, out.dtype)
